# revision 27
# baseline (speedup 1.0000x reference)
"""Trainium2 Bass kernel for the MFPA attention module.

Reference computation (per batch b, with N = H*W = 4096 spatial sites):
    q = Wq @ x_RGB + bq            (CQK=16 channels)
    k = Wk @ x    + bk
    v = Wv @ x    + bv             (C=64 channels)
    energy[i,j] = q_i . k_j
    att = softmax(energy, axis=j)
    out[c,i] = sum_j v[c,j] att[i,j]
    y = lam * out + x

Device strategy (8 NeuronCores): data-parallel over batch (4) x query-row
halves (2).  Each core holds x[b] fully (for K/V and the residual) and its
2048-row query slice, and computes a flash-style streaming softmax so the
4096x4096 energy matrix never leaves PSUM/SBUF.

Host-side weight folding (softmax is shift-invariant, so bk drops out):
    energy[i,j] = (M^T xr_i + bqk) . xf_j    with  M = Wq^T Wk, bqk = Wk^T bq
bqk rides as an extra row of the folded Q-prep weight against an all-ones
row appended to x_RGB.  V is computed on-device from x and wv, with a K=1
ones-row matmul adding the bias row and planting the ones column that makes
the PV matmul also produce the softmax row-sums for free.

Perf design (this TRN2's clock governor parks the PE at 1.2 GHz when the
tensor engine runs as a saturated stream of large matmuls; it grants and
renews 2.4 GHz only for dependency-paced streams):
  - energy matmuls have K=64 contraction, so two j-blocks are row-packed
    into the 128x128 array concurrently.  Softmax is j-permutation
    invariant, so blocks are paired (g, g+16): rows 0-63 of SBUF hold x
    columns 0-2047 and rows 64-127 hold columns 2048-4095 -- no data
    duplication.  This halves energy-stage PE time so the scalar engine
    (exp, 2 j-blocks per ACTIVATE) paces the loop with PE duty ~56% at
    full clock, which keeps the clock grant renewed.
  - inputs are bf16 with 2-4KB rows spread over the sync+scalar HWDGE and
    gpsimd queues; the f32 residual input is dropped (residual re-uses
    bf16 x, well within the 2e-2 gate).
  - the PV accumulator is double-buffered and each chunk's softmax
    renormalization (reciprocal on DVE is slow: 8 cycles/element on one
    lane) is split in halves and deferred into the next chunk, so the PE
    stream never waits on it.
"""

import ml_dtypes
import numpy as np

import concourse.bass as bass
import concourse.mybir as mybir
import concourse.tile as tile_mod
from concourse.vector_clock import ScopedClock

B, C, HH, WW = 4, 64, 64, 64
N = HH * WW          # 4096 spatial sites
NI = N // 2          # query rows per core
CHUNK = 512          # query rows processed per main-loop iteration
NCHUNK = NI // CHUNK
JBLK = 128           # key/value block (PSUM partition dim)
NJ = N // JBLK       # 32 j-blocks
NG = NJ // 2         # 16 lo/hi j-block pairs; lo half = 0-15, hi = 16-31
NCORES = 8
HCOL = NG * JBLK     # 2048: columns per xfd partition-half

# exp grouping: chunk 0 uses (lo,hi) pairs (2 j-blocks per ACTIVATE) -- the
# pattern that earns the clock grant even at 1.2 GHz; later chunks batch 3
# j-blocks per ACTIVATE (10x3+2) to amortize ACT per-call overhead.  The
# interleaved lo/hi order keeps adjacent matmuls row-packable.
_ILV = [jb for t in range(NG) for jb in (t, t + NG)]
_G2 = [[t, t + NG] for t in range(NG)]
_G3 = [_ILV[0:1]] + [_ILV[i : i + 3] for i in range(1, 31, 3)] + [_ILV[31:32]]
# chunk 0: pairs while cold (grant lands ~12 groups in), 3-batches after
_G0 = _G2[:10] + [_ILV[i : i + 3] for i in range(20, 32, 3)]
GROUPS_BY_CHUNK = [_G0] + [_G3] * (NCHUNK - 1)

F32 = mybir.dt.float32
F32R = mybir.dt.float32r
BF16 = mybir.dt.bfloat16


def _patched_drain_and_barrier(self, tick_clock, wait_clock):
    # The walrus build in this container rejects instructions with more than
    # one sync-wait command ("Too many sync wait commands" on the Tile tail
    # drain).  Split the aggregated drain into one drain per semaphore wait.
    nc = self.nc
    drain_inst = nc.sync.drain()
    wait_clock.add_sem_waits(
        drain_inst.ins, ScopedClock({None: tick_clock.global_clock})
    )
    inst = drain_inst.ins
    si = inst.sync_info
    waits = list(si.on_wait or []) if si else []
    if len(waits) > 1:
        si.on_wait = waits[:1]
        for w in waits[1:]:
            extra = nc.sync.drain()
            extra.ins.sync_info = mybir.SyncInfo(on_wait=[w], on_update=[])
    nc.all_engine_barrier()
    popped = nc._tile_sem_poison_stack.pop()
    assert popped is self._sem_poison
    nc.clear_and_free_semaphores(list(self.sems.allocated().values()))
    nc.all_engine_barrier()


tile_mod.TileContext._drain_and_barrier = _patched_drain_and_barrier


def _split_multi_waits(nc):
    # This walrus build accepts at most one sync-wait command per TPB
    # instruction.  Hoist extra waits onto engine NoOps placed just before
    # the instruction (engine executes in order, so semantics are kept).
    for blk in nc.m.functions[0].blocks:
        insts = list(blk.instructions)
        out = []
        changed = False
        for inst in insts:
            si = inst.sync_info
            if si is not None and si.on_wait and len(si.on_wait) > 1:
                waits = list(si.on_wait)
                si.on_wait = waits[-1:]
                for w in waits[:-1]:
                    nop = mybir.InstNoOp(name=nc.get_next_instruction_name())
                    nop.engine = inst.engine
                    nop.sync_info = mybir.SyncInfo(on_wait=[w], on_update=[])
                    out.append(nop)
                changed = True
            out.append(inst)
        if changed:
            blk.instructions = out


def build_bass(split_waits=True):
    nc = bass.Bass()
    NQ = N // 4
    xfp = [
        nc.declare_dram_parameter(f"xf{q}", [C, NQ], BF16, isOutput=False)
        for q in range(4)
    ]
    xq = nc.declare_dram_parameter("xq", [C + 1, NI], BF16, isOutput=False)
    con = nc.declare_dram_parameter("con", [C + 1, 2 * C], BF16, isOutput=False)
    wv2 = nc.declare_dram_parameter("wv2", [2 * C, 66], BF16, isOutput=False)
    bvl = nc.declare_dram_parameter("bvl", [C, 1], F32, isOutput=False)
    onesv = nc.declare_dram_parameter("onesv", [1, C], F32R, isOutput=False)
    ident = nc.declare_dram_parameter("ident", [JBLK, JBLK], BF16, isOutput=False)
    y = nc.declare_dram_parameter("y", [C, NI], F32, isOutput=True)

    EXP = mybir.ActivationFunctionType.Exp
    HC = CHUNK // 2

    with tile_mod.TileContext(nc) as tc:
        with (
            tc.tile_pool(name="sing", bufs=1) as sing,
            tc.tile_pool(name="ppool", bufs=3) as ppool,
            tc.tile_pool(name="ypool", bufs=2) as ypool,
            tc.tile_pool(name="small", bufs=4) as small,
            tc.tile_pool(name="ps_a", bufs=1, space="PSUM") as ps_a,
            tc.tile_pool(name="ps_b", bufs=1, space="PSUM") as ps_b,
            tc.tile_pool(name="ps_pv", bufs=2, space="PSUM") as ps_pv,
        ):
            # ---- SBUF constants built on-device ---------------------------
            ones_sb = sing.tile([1, C], F32R, tag="ones")

            # xfd: x cols 0-2047 in partitions 0-63, cols 2048-4095 in 64-127
            xfd_sb = sing.tile([2 * C, HCOL], BF16, tag="xfd")
            xq_sb = sing.tile([C + 1, NI], BF16, tag="xq")
            con_sb = sing.tile([C + 1, 2 * C], BF16, tag="con")
            wv2_sb = sing.tile([2 * C, 66], BF16, tag="wv2")
            bvl_sb = sing.tile([C, 1], F32, tag="bvl")
            nc.scalar.dma_start(out=con_sb, in_=con[:, :])
            nc.gpsimd.dma_start(out=wv2_sb, in_=wv2[:, :])
            nc.gpsimd.dma_start(out=bvl_sb, in_=bvl[:, :])
            nc.gpsimd.dma_start(out=ones_sb, in_=onesv[:, :])
            id_sb = sing.tile([JBLK, JBLK], BF16, tag="ident")
            nc.gpsimd.dma_start(out=id_sb, in_=ident[:, :])
            nc.sync.dma_start(out=xq_sb, in_=xq[:, :])
            # quarters 0,1 -> partition rows 0:64; quarters 2,3 -> rows 64:128
            nc.sync.dma_start(out=xfd_sb[0:C, 0:NQ], in_=xfp[0][:, :])
            nc.scalar.dma_start(out=xfd_sb[C : 2 * C, 0:NQ], in_=xfp[2][:, :])
            nc.sync.dma_start(out=xfd_sb[0:C, NQ:HCOL], in_=xfp[1][:, :])
            nc.scalar.dma_start(out=xfd_sb[C : 2 * C, NQ:HCOL], in_=xfp[3][:, :])

            # ---- Q prep: qk duplicated into both partition halves ---------
            qk_sbs = []
            for half in range(2):
                pool, ptag = (ps_a, "eta") if half == 0 else (ps_b, "etb")
                qkp = pool.tile(
                    [2 * C, 2, CHUNK], F32, name=f"qkp{half}", tag=ptag,
                    padded_shape=[2 * C, 3, CHUNK],
                )
                for j in range(2):
                    ic = 2 * half + j
                    nc.tensor.matmul(
                        out=qkp[:, j, :],
                        lhsT=con_sb[:, 0 : 2 * C],
                        rhs=xq_sb[:, ic * CHUNK : (ic + 1) * CHUNK],
                        start=True,
                        stop=True,
                    )
                for j in range(2):
                    ic = 2 * half + j
                    qk_sb = sing.tile(
                        [2 * C, CHUNK], BF16, name=f"qk{ic}", tag=f"qk{ic}"
                    )
                    if ic % 2 == 0:
                        nc.scalar.copy(qk_sb, qkp[:, j, :])
                    else:
                        nc.vector.tensor_copy(qk_sb, qkp[:, j, :])
                    qk_sbs.append(qk_sb)

            # ---- V prep: v[j, o] in (j, o) layout -------------------------
            # bv passes through softmax unchanged (attention rows sum to 1),
            # so V carries no bias here; lam*bv is added in the epilogue.
            # The row-sum ones column v[:, :, 64] is planted by memset.
            v_sb = sing.tile([JBLK, NJ, 66], BF16, tag="v")
            nc.vector.memset(v_sb[:, :, 64:65], 1.0)
            for r in range(4):
                vp = ps_b.tile(
                    [JBLK, 8, 66],
                    F32,
                    name=f"vp{r}",
                    tag="etb",
                    padded_shape=[JBLK, 8, 192],
                )
                for k in range(8):
                    jb = 8 * r + k
                    h = C if jb >= NG else 0
                    cb = (jb - NG if jb >= NG else jb) * JBLK
                    nc.tensor.matmul(
                        out=vp[:, k, :],
                        lhsT=xfd_sb[h : h + C, cb : cb + JBLK],
                        rhs=wv2_sb[h : h + C, :],
                        start=True,
                        stop=True,
                    )
                if r % 2 == 0:
                    nc.scalar.copy(
                        v_sb[:, 8 * r : 8 * r + 8, 0:64], vp[:, :, 0:64]
                    )
                else:
                    nc.vector.tensor_copy(
                        v_sb[:, 8 * r : 8 * r + 8, 0:64], vp[:, :, 0:64]
                    )

            # ---- main loop over query chunks ------------------------------
            # the epilogue is split: both slow [1,256] reciprocals issue
            # first (DVE), then each half's broadcast/normalize/store runs at
            # a separate deferral point so the PE never waits on the DVE.
            def make_epi_recips(pv, ic):
                def epi():
                    for hf in range(2):
                        cs = slice(hf * HC, (hf + 1) * HC)
                        r_t = small.tile(
                            [1, HC], F32R, name=f"rt{ic}_{hf}", tag="rt"
                        )
                        with nc.allow_low_precision(reason="f32r recip"):
                            nc.vector.reciprocal(out=r_t, in_=pv[C : C + 1, cs])
                        r_ts.append(r_t)

                return epi

            def make_epi_half(pv, ic, hf):
                def epi():
                    cs = slice(hf * HC, (hf + 1) * HC)
                    isl = slice(ic * CHUNK + hf * HC, ic * CHUNK + (hf + 1) * HC)
                    r_t = r_ts.pop(0)
                    lrb = ps_b.tile(
                        [C, HC],
                        F32,
                        name=f"lrb{ic}_{hf}",
                        tag="etb",
                        padded_shape=[C, 6 * HC],
                    )
                    nc.tensor.matmul(
                        out=lrb, lhsT=ones_sb, rhs=r_t, start=True, stop=True
                    )
                    lrb_sb = small.tile(
                        [C, HC], F32, name=f"lrbsb{ic}_{hf}", tag="lrbsb"
                    )
                    nc.vector.tensor_copy(lrb_sb, lrb)
                    y_t = ypool.tile(
                        [C, HC], F32, name=f"yt{ic}_{hf}", tag="yt"
                    )
                    nc.vector.tensor_tensor(
                        out=y_t, in0=pv[0:C, cs], in1=lrb_sb,
                        op=mybir.AluOpType.mult,
                    )
                    nc.vector.scalar_tensor_tensor(
                        out=y_t,
                        in0=y_t,
                        scalar=bvl_sb,
                        in1=xfd_sb[0:C, isl],
                        op0=mybir.AluOpType.add,
                        op1=mybir.AluOpType.add,
                    )
                    nc.sync.dma_start(out=y[:, isl], in_=y_t)

                return epi

            r_ts = []
            pending = {}
            pre_et = None

            def emit_energy(et, jbs, qk):
                for k, jb in enumerate(jbs):
                    # adjacent lo/hi j-blocks run concurrently in the PE
                    # array's row halves (row-group packing, K=64 each)
                    h = C if jb >= NG else 0
                    cb = (jb - NG if jb >= NG else jb) * JBLK
                    nc.tensor.matmul(
                        out=et[:, k, :],
                        lhsT=xfd_sb[h : h + C, cb : cb + JBLK],
                        rhs=qk[h : h + C, :],
                        start=True,
                        stop=True,
                    )

            for ic in range(NCHUNK):
                qk_sb = qk_sbs[ic]
                groups = GROUPS_BY_CHUNK[ic]
                pv = ps_pv.tile([C + 1, CHUNK], F32, name=f"pv{ic}", tag="pv")
                nk = 0
                for g, jbs in enumerate(groups):
                    if g == 0 and pre_et is not None:
                        et = pre_et  # energies already emitted last chunk
                        pre_et = None
                    else:
                        pool, ptag = (
                            (ps_a, "eta") if g % 2 == 0 else (ps_b, "etb")
                        )
                        et = pool.tile(
                            [JBLK, len(jbs), CHUNK], F32, name=f"et{ic}_{g}",
                            tag=ptag, padded_shape=[JBLK, 3, CHUNK],
                        )
                        emit_energy(et, jbs, qk_sb)
                    if g == len(groups) - 1 and ic < NCHUNK - 1:
                        # pre-emit next chunk's first energies so the scalar
                        # engine never bubbles across the chunk boundary
                        njbs = GROUPS_BY_CHUNK[ic + 1][0]
                        pre_et = ps_a.tile(
                            [JBLK, len(njbs), CHUNK], F32,
                            name=f"et{ic + 1}_0", tag="eta",
                            padded_shape=[JBLK, 3, CHUNK],
                        )
                        emit_energy(pre_et, njbs, qk_sbs[ic + 1])
                    if g in pending:
                        pending.pop(g)()
                    p_t = ppool.tile(
                        [JBLK, len(jbs), CHUNK], BF16, name=f"pt{ic}_{g}",
                        tag="pt", padded_shape=[JBLK, 3, CHUNK],
                    )
                    nc.scalar.activation(out=p_t, in_=et, func=EXP)
                    for k, jb in enumerate(jbs):
                        nc.tensor.matmul(
                            out=pv,
                            lhsT=v_sb[:, jb, 0:65],
                            rhs=p_t[:, k, :],
                            start=(nk == 0),
                            stop=(nk == NJ - 1),
                        )
                        nk += 1

                if ic < NCHUNK - 1:
                    pending = {
                        3: make_epi_recips(pv, ic),
                        5: make_epi_half(pv, ic, 0),
                        9: make_epi_half(pv, ic, 1),
                    }
                else:
                    # transposed tail: PE-transpose pv so the reciprocal runs
                    # across all 128 lanes, then transpose back
                    isl = slice(ic * CHUNK, (ic + 1) * CHUNK)
                    pv_sb = sing.tile([C + 1, CHUNK], BF16, tag="pvsb")
                    nc.scalar.copy(pv_sb, pv)
                    pvT = ps_a.tile(
                        [JBLK, 4, C + 1], BF16, tag="eta",
                        padded_shape=[JBLK, 4, 384],
                    )
                    for q in range(4):
                        nc.tensor.transpose(
                            out=pvT[:, q, :],
                            in_=pv_sb[:, q * JBLK : (q + 1) * JBLK],
                            identity=id_sb[0 : C + 1, 0 : C + 1],
                        )
                    r_sb = sing.tile([JBLK, 4], F32, tag="rsb")
                    nc.vector.reciprocal(out=r_sb, in_=pvT[:, :, C])
                    yT_sb = sing.tile([JBLK, 4, C], BF16, tag="ytt")
                    for q in range(4):
                        nc.vector.tensor_scalar_mul(
                            yT_sb[:, q, :], pvT[:, q, 0:C], r_sb[:, q : q + 1]
                        )
                    yb = ps_b.tile(
                        [C, CHUNK], BF16, tag="etb",
                        padded_shape=[C, 6 * HC],
                    )
                    for q in range(4):
                        nc.tensor.transpose(
                            out=yb[:, q * JBLK : (q + 1) * JBLK],
                            in_=yT_sb[:, q, :],
                            identity=id_sb,
                        )
                    y_t = ypool.tile([C, CHUNK], F32, name="ytfin", tag="yt2")
                    for hf in range(2):
                        cs = slice(hf * HC, (hf + 1) * HC)
                        ysl = slice(
                            ic * CHUNK + hf * HC, ic * CHUNK + (hf + 1) * HC
                        )
                        nc.vector.scalar_tensor_tensor(
                            out=y_t[:, cs],
                            in0=yb[:, cs],
                            scalar=bvl_sb,
                            in1=xfd_sb[0:C, ysl],
                            op0=mybir.AluOpType.add,
                            op1=mybir.AluOpType.add,
                        )
                        nc.sync.dma_start(out=y[:, ysl], in_=y_t[:, cs])

    if split_waits:
        _split_multi_waits(nc)
    return nc


_CACHE = {}


def kernel(**inputs):
    x = np.ascontiguousarray(np.asarray(inputs["x"], dtype=np.float32))
    x_RGB = np.ascontiguousarray(np.asarray(inputs["x_RGB"], dtype=np.float32))
    Wq = np.asarray(inputs["Wq"], dtype=np.float32)
    bq = np.asarray(inputs["bq"], dtype=np.float32)
    Wk = np.asarray(inputs["Wk"], dtype=np.float32)
    Wv = np.asarray(inputs["Wv"], dtype=np.float32)
    bv = np.asarray(inputs["bv"], dtype=np.float32)
    lam = np.asarray(inputs["lam"], dtype=np.float32)

    M = (Wq.T.astype(np.float64) @ Wk.astype(np.float64)).astype(np.float32)
    bqk = (Wk.T.astype(np.float64) @ bq.astype(np.float64)).astype(np.float32)
    lamf = float(lam.reshape(-1)[0])

    con = np.zeros((C + 1, 2 * C), np.float32)
    con[:C, 0:C] = M
    con[C, 0:C] = bqk
    con[:C, C : 2 * C] = M          # duplicated for the upper-row-half pack
    con[C, C : 2 * C] = bqk
    con_bf = con.astype(ml_dtypes.bfloat16)

    wv2 = np.zeros((2 * C, 66), np.float32)
    wv2[:C, :C] = Wv.T * lamf
    wv2[C:, :C] = Wv.T * lamf
    bvl_h = (bv * lamf).reshape(C, 1).astype(np.float32)

    xf3 = x.reshape(B, C, N)
    xr3 = x_RGB.reshape(B, C, N)

    if "nc" not in _CACHE:
        _CACHE["nc"] = build_bass()
    nc = _CACHE["nc"]

    NQ = N // 4
    in_maps = []
    for core in range(NCORES):
        b, ih = core >> 1, core & 1
        xf_own = np.empty((C, N), np.float32)
        # own query half first (static residual slice), other half after
        xf_own[:, :NI] = xf3[b][:, ih * NI : (ih + 1) * NI]
        xf_own[:, NI:] = xf3[b][:, (1 - ih) * NI : (2 - ih) * NI]
        xf_bf = xf_own.astype(ml_dtypes.bfloat16)
        xq_aug = np.empty((C + 1, NI), np.float32)
        xq_aug[:C] = xr3[b][:, ih * NI : (ih + 1) * NI]
        xq_aug[C] = 1.0
        m = {
            f"xf{q}": np.ascontiguousarray(xf_bf[:, q * NQ : (q + 1) * NQ])
            for q in range(4)
        }
        m["xq"] = xq_aug.astype(ml_dtypes.bfloat16)
        m["con"] = con_bf
        m["wv2"] = wv2.astype(ml_dtypes.bfloat16)
        m["bvl"] = bvl_h
        m["onesv"] = np.ones((1, C), np.float32)
        m["ident"] = np.eye(JBLK, dtype=np.float32).astype(ml_dtypes.bfloat16)
        in_maps.append(m)

    from concourse.bass_utils import run_bass_kernel_spmd

    res = run_bass_kernel_spmd(nc, in_maps, list(range(NCORES)))

    out = np.empty((B, C, N), np.float32)
    for core in range(NCORES):
        b, ih = core >> 1, core & 1
        out[b][:, ih * NI : (ih + 1) * NI] = res.results[core]["y"]
    return out.reshape(B, C, HH, WW)


# revision 28
# speedup vs baseline: 1.0154x; 1.0154x over previous
"""Trainium2 Bass kernel for the MFPA attention module.

Reference computation (per batch b, with N = H*W = 4096 spatial sites):
    q = Wq @ x_RGB + bq            (CQK=16 channels)
    k = Wk @ x    + bk
    v = Wv @ x    + bv             (C=64 channels)
    energy[i,j] = q_i . k_j
    att = softmax(energy, axis=j)
    out[c,i] = sum_j v[c,j] att[i,j]
    y = lam * out + x

Device strategy (8 NeuronCores): data-parallel over batch (4) x query-row
halves (2).  Each core holds x[b] fully (for K/V and the residual) and its
2048-row query slice, and computes a flash-style streaming softmax so the
4096x4096 energy matrix never leaves PSUM/SBUF.

Host-side weight folding (softmax is shift-invariant, so bk drops out):
    energy[i,j] = (M^T xr_i + bqk) . xf_j    with  M = Wq^T Wk, bqk = Wk^T bq
bqk rides as an extra row of the folded Q-prep weight against an all-ones
row appended to x_RGB.  V carries no bias on-device: bv passes through
softmax unchanged (attention rows sum to 1), so lam*bv is added in the
epilogue.  A memset ones column in v makes the PV matmul also produce the
softmax row-sums for free.

Perf design (this TRN2's clock governor parks the PE at 1.2 GHz when the
tensor engine runs as a saturated stream of large matmuls; it grants and
renews 2.4 GHz only for dependency-paced streams):
  - energy matmuls have K=64 contraction, so two j-blocks are row-packed
    into the 128x128 array concurrently.  Softmax is j-permutation
    invariant, so blocks are paired (g, g+16): rows 0-63 of SBUF hold x
    columns 0-2047 and rows 64-127 hold columns 2048-4095 -- no data
    duplication.  This halves energy-stage PE time so the scalar engine
    (exp, 2 j-blocks per ACTIVATE) paces the loop with PE duty ~56% at
    full clock, which keeps the clock grant renewed.
  - inputs are bf16 with 2-4KB rows spread over the sync+scalar HWDGE and
    gpsimd queues; the f32 residual input is dropped (residual re-uses
    bf16 x, well within the 2e-2 gate).
  - the PV accumulator is double-buffered and each chunk's softmax
    renormalization (reciprocal on DVE is slow: 8 cycles/element on one
    lane) is split in halves and deferred into the next chunk, so the PE
    stream never waits on it.  The final chunk instead uses a PE-transpose
    epilogue so its reciprocal runs across all 128 lanes (~158ns).
"""

import ml_dtypes
import numpy as np

import concourse.bass as bass
import concourse.mybir as mybir
import concourse.tile as tile_mod
from concourse.vector_clock import ScopedClock

B, C, HH, WW = 4, 64, 64, 64
N = HH * WW          # 4096 spatial sites
NI = N // 2          # query rows per core
CHUNK = 512          # query rows processed per main-loop iteration
NCHUNK = NI // CHUNK
JBLK = 128           # key/value block (PSUM partition dim)
NJ = N // JBLK       # 32 j-blocks
NG = NJ // 2         # 16 lo/hi j-block pairs; lo half = 0-15, hi = 16-31
NCORES = 8
HCOL = NG * JBLK     # 2048: columns per xfd partition-half

# exp grouping: chunk 0 uses (lo,hi) pairs (2 j-blocks per ACTIVATE) -- the
# pattern that earns the clock grant even at 1.2 GHz; later chunks batch 3
# j-blocks per ACTIVATE (10x3+2) to amortize ACT per-call overhead.  The
# interleaved lo/hi order keeps adjacent matmuls row-packable.
_ILV = [jb for t in range(NG) for jb in (t, t + NG)]
_G2 = [[t, t + NG] for t in range(NG)]
_G3 = [_ILV[0:1]] + [_ILV[i : i + 3] for i in range(1, 31, 3)] + [_ILV[31:32]]
# chunk 0: pairs while cold (grant lands ~12 groups in), 3-batches after
_G0 = _G2[:10] + [_ILV[i : i + 3] for i in range(20, 32, 3)]
GROUPS_BY_CHUNK = [_G0] + [_G3] * (NCHUNK - 1)

F32 = mybir.dt.float32
F32R = mybir.dt.float32r
BF16 = mybir.dt.bfloat16


def _patched_drain_and_barrier(self, tick_clock, wait_clock):
    # The walrus build in this container rejects instructions with more than
    # one sync-wait command ("Too many sync wait commands" on the Tile tail
    # drain).  Split the aggregated drain into one drain per semaphore wait.
    nc = self.nc
    drain_inst = nc.sync.drain()
    wait_clock.add_sem_waits(
        drain_inst.ins, ScopedClock({None: tick_clock.global_clock})
    )
    inst = drain_inst.ins
    si = inst.sync_info
    waits = list(si.on_wait or []) if si else []
    if len(waits) > 1:
        si.on_wait = waits[:1]
        for w in waits[1:]:
            extra = nc.sync.drain()
            extra.ins.sync_info = mybir.SyncInfo(on_wait=[w], on_update=[])
    nc.all_engine_barrier()
    popped = nc._tile_sem_poison_stack.pop()
    assert popped is self._sem_poison
    nc.clear_and_free_semaphores(list(self.sems.allocated().values()))
    nc.all_engine_barrier()


tile_mod.TileContext._drain_and_barrier = _patched_drain_and_barrier


def _split_multi_waits(nc):
    # This walrus build accepts at most one sync-wait command per TPB
    # instruction.  Hoist extra waits onto engine NoOps placed just before
    # the instruction (engine executes in order, so semantics are kept).
    for blk in nc.m.functions[0].blocks:
        insts = list(blk.instructions)
        out = []
        changed = False
        for inst in insts:
            si = inst.sync_info
            if si is not None and si.on_wait and len(si.on_wait) > 1:
                waits = list(si.on_wait)
                si.on_wait = waits[-1:]
                for w in waits[:-1]:
                    nop = mybir.InstNoOp(name=nc.get_next_instruction_name())
                    nop.engine = inst.engine
                    nop.sync_info = mybir.SyncInfo(on_wait=[w], on_update=[])
                    out.append(nop)
                changed = True
            out.append(inst)
        if changed:
            blk.instructions = out


def build_bass(split_waits=True):
    nc = bass.Bass()
    NQ = N // 4
    xfp = [
        nc.declare_dram_parameter(f"xf{q}", [C, NQ], BF16, isOutput=False)
        for q in range(4)
    ]
    xq = nc.declare_dram_parameter("xq", [C + 1, NI], BF16, isOutput=False)
    con = nc.declare_dram_parameter("con", [C + 1, 2 * C], BF16, isOutput=False)
    wv2 = nc.declare_dram_parameter("wv2", [2 * C, 66], BF16, isOutput=False)
    bvl = nc.declare_dram_parameter("bvl", [C, 1], F32, isOutput=False)
    onesv = nc.declare_dram_parameter("onesv", [1, C], F32R, isOutput=False)
    ident = nc.declare_dram_parameter("ident", [JBLK, JBLK], BF16, isOutput=False)
    y = nc.declare_dram_parameter("y", [C, NI], F32, isOutput=True)

    EXP = mybir.ActivationFunctionType.Exp
    HC = CHUNK // 2

    with tile_mod.TileContext(nc) as tc:
        with (
            tc.tile_pool(name="sing", bufs=1) as sing,
            tc.tile_pool(name="ppool", bufs=3) as ppool,
            tc.tile_pool(name="ypool", bufs=2) as ypool,
            tc.tile_pool(name="small", bufs=4) as small,
            tc.tile_pool(name="ps_a", bufs=1, space="PSUM") as ps_a,
            tc.tile_pool(name="ps_b", bufs=1, space="PSUM") as ps_b,
            tc.tile_pool(name="ps_pv", bufs=2, space="PSUM") as ps_pv,
        ):
            # ---- SBUF constants built on-device ---------------------------
            ones_sb = sing.tile([1, C], F32R, tag="ones")

            # xfd: x cols 0-2047 in partitions 0-63, cols 2048-4095 in 64-127
            xfd_sb = sing.tile([2 * C, HCOL], BF16, tag="xfd")
            xq_sb = sing.tile([C + 1, NI], BF16, tag="xq")
            con_sb = sing.tile([C + 1, 2 * C], BF16, tag="con")
            wv2_sb = sing.tile([2 * C, 66], BF16, tag="wv2")
            bvl_sb = sing.tile([C, 1], F32, tag="bvl")
            nc.scalar.dma_start(out=con_sb, in_=con[:, :])
            nc.gpsimd.dma_start(out=wv2_sb, in_=wv2[:, :])
            nc.gpsimd.dma_start(out=bvl_sb, in_=bvl[:, :])
            nc.gpsimd.dma_start(out=ones_sb, in_=onesv[:, :])
            id_sb = sing.tile([JBLK, JBLK], BF16, tag="ident")
            nc.gpsimd.dma_start(out=id_sb, in_=ident[:, :])
            nc.sync.dma_start(out=xq_sb, in_=xq[:, :])
            # quarters 0,1 -> partition rows 0:64; quarters 2,3 -> rows 64:128
            nc.sync.dma_start(out=xfd_sb[0:C, 0:NQ], in_=xfp[0][:, :])
            nc.scalar.dma_start(out=xfd_sb[C : 2 * C, 0:NQ], in_=xfp[2][:, :])
            nc.sync.dma_start(out=xfd_sb[0:C, NQ:HCOL], in_=xfp[1][:, :])
            nc.scalar.dma_start(out=xfd_sb[C : 2 * C, NQ:HCOL], in_=xfp[3][:, :])

            # ---- Q prep: qk duplicated into both partition halves ---------
            qk_sbs = []
            for half in range(2):
                pool, ptag = (ps_a, "eta") if half == 0 else (ps_b, "etb")
                qkp = pool.tile(
                    [2 * C, 2, CHUNK], F32, name=f"qkp{half}", tag=ptag,
                    padded_shape=[2 * C, 3, CHUNK],
                )
                for j in range(2):
                    ic = 2 * half + j
                    nc.tensor.matmul(
                        out=qkp[:, j, :],
                        lhsT=con_sb[:, 0 : 2 * C],
                        rhs=xq_sb[:, ic * CHUNK : (ic + 1) * CHUNK],
                        start=True,
                        stop=True,
                    )
                for j in range(2):
                    ic = 2 * half + j
                    qk_sb = sing.tile(
                        [2 * C, CHUNK], BF16, name=f"qk{ic}", tag=f"qk{ic}"
                    )
                    if ic % 2 == 0:
                        nc.scalar.copy(qk_sb, qkp[:, j, :])
                    else:
                        nc.vector.tensor_copy(qk_sb, qkp[:, j, :])
                    qk_sbs.append(qk_sb)

            # ---- V prep: v[j, o] in (j, o) layout -------------------------
            # bv passes through softmax unchanged (attention rows sum to 1),
            # so V carries no bias here; lam*bv is added in the epilogue.
            # The row-sum ones column v[:, :, 64] is planted by memset.
            v_sb = sing.tile([JBLK, NJ, 66], BF16, tag="v")
            nc.vector.memset(v_sb[:, :, 64:65], 1.0)
            for r in range(4):
                vp = ps_b.tile(
                    [JBLK, 8, 66],
                    F32,
                    name=f"vp{r}",
                    tag="etb",
                    padded_shape=[JBLK, 8, 192],
                )
                for k in range(8):
                    jb = 8 * r + k
                    h = C if jb >= NG else 0
                    cb = (jb - NG if jb >= NG else jb) * JBLK
                    nc.tensor.matmul(
                        out=vp[:, k, :],
                        lhsT=xfd_sb[h : h + C, cb : cb + JBLK],
                        rhs=wv2_sb[h : h + C, :],
                        start=True,
                        stop=True,
                    )
                if r % 2 == 0:
                    nc.scalar.copy(
                        v_sb[:, 8 * r : 8 * r + 8, 0:64], vp[:, :, 0:64]
                    )
                else:
                    nc.vector.tensor_copy(
                        v_sb[:, 8 * r : 8 * r + 8, 0:64], vp[:, :, 0:64]
                    )

            # ---- main loop over query chunks ------------------------------
            # the epilogue is split: both slow [1,256] reciprocals issue
            # first (DVE), then each half's broadcast/normalize/store runs at
            # a separate deferral point so the PE never waits on the DVE.
            def make_epi_recips(pv, ic):
                def epi():
                    for hf in range(2):
                        cs = slice(hf * HC, (hf + 1) * HC)
                        r_t = small.tile(
                            [1, HC], F32R, name=f"rt{ic}_{hf}", tag="rt"
                        )
                        with nc.allow_low_precision(reason="f32r recip"):
                            nc.vector.reciprocal(out=r_t, in_=pv[C : C + 1, cs])
                        r_ts.append(r_t)

                return epi

            def make_epi_half(pv, ic, hf):
                def epi():
                    cs = slice(hf * HC, (hf + 1) * HC)
                    isl = slice(ic * CHUNK + hf * HC, ic * CHUNK + (hf + 1) * HC)
                    r_t = r_ts.pop(0)
                    lrb = ps_b.tile(
                        [C, HC],
                        F32,
                        name=f"lrb{ic}_{hf}",
                        tag="etb",
                        padded_shape=[C, 6 * HC],
                    )
                    nc.tensor.matmul(
                        out=lrb, lhsT=ones_sb, rhs=r_t, start=True, stop=True
                    )
                    lrb_sb = small.tile(
                        [C, HC], F32, name=f"lrbsb{ic}_{hf}", tag="lrbsb"
                    )
                    nc.vector.tensor_copy(lrb_sb, lrb)
                    y_t = ypool.tile(
                        [C, HC], F32, name=f"yt{ic}_{hf}", tag="yt"
                    )
                    nc.vector.tensor_tensor(
                        out=y_t, in0=pv[0:C, cs], in1=lrb_sb,
                        op=mybir.AluOpType.mult,
                    )
                    nc.vector.scalar_tensor_tensor(
                        out=y_t,
                        in0=y_t,
                        scalar=bvl_sb,
                        in1=xfd_sb[0:C, isl],
                        op0=mybir.AluOpType.add,
                        op1=mybir.AluOpType.add,
                    )
                    nc.sync.dma_start(out=y[:, isl], in_=y_t)

                return epi

            r_ts = []
            pending = {}
            pre_et = None

            def emit_energy(et, jbs, qk):
                for k, jb in enumerate(jbs):
                    # adjacent lo/hi j-blocks run concurrently in the PE
                    # array's row halves (row-group packing, K=64 each)
                    h = C if jb >= NG else 0
                    cb = (jb - NG if jb >= NG else jb) * JBLK
                    nc.tensor.matmul(
                        out=et[:, k, :],
                        lhsT=xfd_sb[h : h + C, cb : cb + JBLK],
                        rhs=qk[h : h + C, :],
                        start=True,
                        stop=True,
                    )

            for ic in range(NCHUNK):
                qk_sb = qk_sbs[ic]
                groups = GROUPS_BY_CHUNK[ic]
                pv = ps_pv.tile([C + 1, CHUNK], F32, name=f"pv{ic}", tag="pv")
                nk = 0
                for g, jbs in enumerate(groups):
                    if g == 0 and pre_et is not None:
                        et = pre_et  # energies already emitted last chunk
                        pre_et = None
                    else:
                        pool, ptag = (
                            (ps_a, "eta") if g % 2 == 0 else (ps_b, "etb")
                        )
                        et = pool.tile(
                            [JBLK, len(jbs), CHUNK], F32, name=f"et{ic}_{g}",
                            tag=ptag, padded_shape=[JBLK, 3, CHUNK],
                        )
                        emit_energy(et, jbs, qk_sb)
                    if g == len(groups) - 1 and ic < NCHUNK - 1:
                        # pre-emit next chunk's first energies so the scalar
                        # engine never bubbles across the chunk boundary
                        njbs = GROUPS_BY_CHUNK[ic + 1][0]
                        pre_et = ps_a.tile(
                            [JBLK, len(njbs), CHUNK], F32,
                            name=f"et{ic + 1}_0", tag="eta",
                            padded_shape=[JBLK, 3, CHUNK],
                        )
                        emit_energy(pre_et, njbs, qk_sbs[ic + 1])
                    if g in pending:
                        pending.pop(g)()
                    p_t = ppool.tile(
                        [JBLK, len(jbs), CHUNK], BF16, name=f"pt{ic}_{g}",
                        tag="pt", padded_shape=[JBLK, 3, CHUNK],
                    )
                    nc.scalar.activation(out=p_t, in_=et, func=EXP)
                    for k, jb in enumerate(jbs):
                        nc.tensor.matmul(
                            out=pv,
                            lhsT=v_sb[:, jb, 0:65],
                            rhs=p_t[:, k, :],
                            start=(nk == 0),
                            stop=(nk == NJ - 1),
                        )
                        nk += 1

                if ic < NCHUNK - 1:
                    pending = {
                        3: make_epi_recips(pv, ic),
                        5: make_epi_half(pv, ic, 0),
                        9: make_epi_half(pv, ic, 1),
                    }
                else:
                    # transposed tail: PE-transpose pv so the reciprocal runs
                    # across all 128 lanes, then transpose back
                    isl = slice(ic * CHUNK, (ic + 1) * CHUNK)
                    pv_sb = sing.tile([C + 1, CHUNK], BF16, tag="pvsb")
                    nc.scalar.copy(pv_sb, pv)
                    pvT = ps_a.tile(
                        [JBLK, 4, C + 1], BF16, tag="eta",
                        padded_shape=[JBLK, 4, 384],
                    )
                    for q in range(4):
                        nc.tensor.transpose(
                            out=pvT[:, q, :],
                            in_=pv_sb[:, q * JBLK : (q + 1) * JBLK],
                            identity=id_sb[0 : C + 1, 0 : C + 1],
                        )
                    r_sb = sing.tile([JBLK, 4], F32, tag="rsb")
                    nc.vector.reciprocal(out=r_sb, in_=pvT[:, :, C])
                    yT_sb = sing.tile([JBLK, 4, C], BF16, tag="ytt")
                    for q in range(4):
                        nc.vector.tensor_scalar_mul(
                            yT_sb[:, q, :], pvT[:, q, 0:C], r_sb[:, q : q + 1]
                        )
                    yb = ps_b.tile(
                        [C, CHUNK], BF16, tag="etb",
                        padded_shape=[C, 6 * HC],
                    )
                    for q in range(4):
                        nc.tensor.transpose(
                            out=yb[:, q * JBLK : (q + 1) * JBLK],
                            in_=yT_sb[:, q, :],
                            identity=id_sb,
                        )
                    y_t = ypool.tile([C, CHUNK], F32, name="ytfin", tag="yt2")
                    for hf in range(2):
                        cs = slice(hf * HC, (hf + 1) * HC)
                        ysl = slice(
                            ic * CHUNK + hf * HC, ic * CHUNK + (hf + 1) * HC
                        )
                        nc.vector.scalar_tensor_tensor(
                            out=y_t[:, cs],
                            in0=yb[:, cs],
                            scalar=bvl_sb,
                            in1=xfd_sb[0:C, ysl],
                            op0=mybir.AluOpType.add,
                            op1=mybir.AluOpType.add,
                        )
                        nc.sync.dma_start(out=y[:, ysl], in_=y_t[:, cs])

    if split_waits:
        _split_multi_waits(nc)
    return nc


_CACHE = {}


def kernel(**inputs):
    x = np.ascontiguousarray(np.asarray(inputs["x"], dtype=np.float32))
    x_RGB = np.ascontiguousarray(np.asarray(inputs["x_RGB"], dtype=np.float32))
    Wq = np.asarray(inputs["Wq"], dtype=np.float32)
    bq = np.asarray(inputs["bq"], dtype=np.float32)
    Wk = np.asarray(inputs["Wk"], dtype=np.float32)
    Wv = np.asarray(inputs["Wv"], dtype=np.float32)
    bv = np.asarray(inputs["bv"], dtype=np.float32)
    lam = np.asarray(inputs["lam"], dtype=np.float32)

    M = (Wq.T.astype(np.float64) @ Wk.astype(np.float64)).astype(np.float32)
    bqk = (Wk.T.astype(np.float64) @ bq.astype(np.float64)).astype(np.float32)
    lamf = float(lam.reshape(-1)[0])

    con = np.zeros((C + 1, 2 * C), np.float32)
    con[:C, 0:C] = M
    con[C, 0:C] = bqk
    con[:C, C : 2 * C] = M          # duplicated for the upper-row-half pack
    con[C, C : 2 * C] = bqk
    con_bf = con.astype(ml_dtypes.bfloat16)

    wv2 = np.zeros((2 * C, 66), np.float32)
    wv2[:C, :C] = Wv.T * lamf
    wv2[C:, :C] = Wv.T * lamf
    bvl_h = (bv * lamf).reshape(C, 1).astype(np.float32)

    xf3 = x.reshape(B, C, N)
    xr3 = x_RGB.reshape(B, C, N)

    if "nc" not in _CACHE:
        _CACHE["nc"] = build_bass()
    nc = _CACHE["nc"]

    NQ = N // 4
    in_maps = []
    for core in range(NCORES):
        b, ih = core >> 1, core & 1
        xf_own = np.empty((C, N), np.float32)
        # own query half first (static residual slice), other half after
        xf_own[:, :NI] = xf3[b][:, ih * NI : (ih + 1) * NI]
        xf_own[:, NI:] = xf3[b][:, (1 - ih) * NI : (2 - ih) * NI]
        xf_bf = xf_own.astype(ml_dtypes.bfloat16)
        xq_aug = np.empty((C + 1, NI), np.float32)
        xq_aug[:C] = xr3[b][:, ih * NI : (ih + 1) * NI]
        xq_aug[C] = 1.0
        m = {
            f"xf{q}": np.ascontiguousarray(xf_bf[:, q * NQ : (q + 1) * NQ])
            for q in range(4)
        }
        m["xq"] = xq_aug.astype(ml_dtypes.bfloat16)
        m["con"] = con_bf
        m["wv2"] = wv2.astype(ml_dtypes.bfloat16)
        m["bvl"] = bvl_h
        m["onesv"] = np.ones((1, C), np.float32)
        m["ident"] = np.eye(JBLK, dtype=np.float32).astype(ml_dtypes.bfloat16)
        in_maps.append(m)

    from concourse.bass_utils import run_bass_kernel_spmd

    res = run_bass_kernel_spmd(nc, in_maps, list(range(NCORES)))

    out = np.empty((B, C, N), np.float32)
    for core in range(NCORES):
        b, ih = core >> 1, core & 1
        out[b][:, ih * NI : (ih + 1) * NI] = res.results[core]["y"]
    return out.reshape(B, C, HH, WW)
